# revision 4
# baseline (speedup 1.0000x reference)
"""Trainium2 kernel for nn_CuriosityEngine.

The reference broadcasts x to 32 identical "action replicas" before the two
EntangledBottleneck passes, so the 32 per-action entropies are bitwise
identical and argmax over them is always 0, for any input.  The only
input-dependent output is `surprise`:

    state_mean = x.mean(axis=1)                      # (B, D, 2): reads all of x
    surprise   = relu(state_mean.flat @ W1 + b1) @ W2 + b2

The memory roofline is one pass over x (48 MB -> 6 MB/core on 8 cores).

Sharding: x.reshape(8192, 1536) is split into 8 contiguous 1024-row blocks
(core c's rows all belong to batch b = c // 2), so shards are zero-copy views
and the host just adds core-pair partials.

Per-core kernel (raw Bass, hand-scheduled; ~24.8 us cost-model, ~12 us/body
measured on HW via replication slope — DMA-bound):
  - 16 HWDGE DMAs on one queue: 8 tiles split into L (1024 cols) then
    R (512 cols) streams; in-order completion per queue.
  - DVE sequentially accumulates tiles into accL/accR as they land (the adds
    hide under the DMA stream; per-tile semaphores + a retired-add counter
    keep every semaphore edge deterministic).
  - GpSimd partition_all_reduce folds the 128 partitions; the big L reduce
    hides under the R DMA stream, so the tail after the last byte is just
    small-R add -> small-R reduce -> one 6 KB out-DMA.
Host: sum core-pair partials in f64, divide by T, run the 1.5 MFLOP MLP.
"""

import contextlib

import numpy as np

import concourse.bass as bass
import concourse.bacc as bacc
import concourse.bass_isa as bass_isa
import concourse.mybir as mybir
from concourse import library_config
from concourse.bass_utils import run_bass_kernel_spmd

N_CORES = 8
B, T, D = 4, 2048, 768
F = 2 * D            # 1536
ROWS = (B * T) // N_CORES  # 1024 rows per core
P = 128
N_TILES = ROWS // P  # 8
N_ADDS = 2 * (N_TILES - 1)
HL = 1024            # left-half columns (bulk)
HR = F - HL          # 512 right-half columns (tail chain runs on R)
F32 = mybir.dt.float32

TRACE = False
LAST_RESULTS = None
_NC = None


def _build():
    nc = bacc.Bacc("TRN2", num_devices=N_CORES)
    xs = nc.dram_tensor("xs", [ROWS, F], F32, kind="ExternalInput")
    out = nc.dram_tensor("partial", [1, F], F32, kind="ExternalOutput")

    with contextlib.ExitStack() as ctx:
        xtL = [ctx.enter_context(nc.sbuf_tensor(f"xtL{i}", [P, HL], F32))
               for i in range(N_TILES)]
        xtR = [ctx.enter_context(nc.sbuf_tensor(f"xtR{i}", [P, HR], F32))
               for i in range(N_TILES)]
        accL = ctx.enter_context(nc.sbuf_tensor("accL", [P, HL], F32))
        accR = ctx.enter_context(nc.sbuf_tensor("accR", [P, HR], F32))
        red = ctx.enter_context(nc.sbuf_tensor("red", [P, F], F32))
        semL = [ctx.enter_context(nc.semaphore(f"semL{i}"))
                for i in range(N_TILES)]
        semR = [ctx.enter_context(nc.semaphore(f"semR{i}"))
                for i in range(N_TILES)]
        dve_sem = ctx.enter_context(nc.semaphore("dve_sem"))
        red_sem = ctx.enter_context(nc.semaphore("red_sem"))
        out_sem = ctx.enter_context(nc.semaphore("out_sem"))
        block = ctx.enter_context(nc.Block())

        @block.sync
        def _(sync):
            for i in range(N_TILES):
                sync.dma_start(
                    xtL[i][:], xs[i * P:(i + 1) * P, 0:HL]
                ).then_inc(semL[i], 16)
            for i in range(N_TILES):
                sync.dma_start(
                    xtR[i][:], xs[i * P:(i + 1) * P, HL:F]
                ).then_inc(semR[i], 16)
            sync.wait_ge(red_sem, 2)
            sync.dma_start(out[0:1, :], red[0:1, :]).then_inc(out_sem, 16)
            sync.wait_ge(out_sem, 16)

        @block.vector
        def _(vector):
            # L chain then R chain; dve_sem counts retired adds (the DVE
            # pipeline has no RAW interlock, so each add waits for its
            # predecessor to retire).
            k = 0
            vector.wait_ge(semL[0], 16)
            vector.wait_ge(semL[1], 16)
            vector.tensor_add(accL[:], xtL[0][:], xtL[1][:]).then_inc(dve_sem, 1)
            k += 1
            for i in range(2, N_TILES):
                vector.wait_ge(semL[i], 16)
                vector.wait_ge(dve_sem, k)
                vector.tensor_add(accL[:], accL[:], xtL[i][:]).then_inc(dve_sem, 1)
                k += 1
            vector.wait_ge(semR[0], 16)
            vector.wait_ge(semR[1], 16)
            vector.tensor_add(accR[:], xtR[0][:], xtR[1][:]).then_inc(dve_sem, 1)
            k += 1
            for i in range(2, N_TILES):
                vector.wait_ge(semR[i], 16)
                vector.wait_ge(dve_sem, k)
                vector.tensor_add(accR[:], accR[:], xtR[i][:]).then_inc(dve_sem, 1)
                k += 1

        @block.gpsimd
        def _(gpsimd):
            gpsimd.load_library(library_config.mlp)
            gpsimd.wait_ge(dve_sem, N_TILES - 1)
            gpsimd.partition_all_reduce(
                red[:, 0:HL], accL[:], channels=P,
                reduce_op=bass_isa.ReduceOp.add,
            ).then_inc(red_sem, 1)
            gpsimd.wait_ge(dve_sem, N_ADDS)
            gpsimd.partition_all_reduce(
                red[:, HL:F], accR[:], channels=P,
                reduce_op=bass_isa.ReduceOp.add,
            ).then_inc(red_sem, 1)

    nc.compile()
    return nc


def _get_nc():
    global _NC
    if _NC is None:
        _NC = _build()
    return _NC


def kernel(x, W1, b1, W2, b2, Wd, bd, Wu, bu):
    global LAST_RESULTS
    x = np.ascontiguousarray(np.asarray(x, dtype=np.float32))
    W1 = np.asarray(W1, dtype=np.float32)
    b1 = np.asarray(b1, dtype=np.float32)
    W2 = np.asarray(W2, dtype=np.float32)
    b2 = np.asarray(b2, dtype=np.float32)

    x_flat = x.reshape(B * T, F)
    in_maps = [{"xs": x_flat[c * ROWS:(c + 1) * ROWS]} for c in range(N_CORES)]

    nc = _get_nc()
    res = run_bass_kernel_spmd(nc, in_maps, list(range(N_CORES)), trace=TRACE)
    LAST_RESULTS = res

    partials = np.stack([r["partial"][0] for r in res.results])  # (8, 1536)
    sums = partials.astype(np.float64).reshape(B, 2, F).sum(axis=1)
    state_mean = (sums / T).astype(np.float32)                   # (B, 1536)

    h = np.maximum(state_mean @ W1 + b1, 0.0)
    surprise = (h @ W2 + b2).astype(np.float32).reshape(B)

    best_action_idx = np.array(0, dtype=np.int32)
    return surprise, best_action_idx


# revision 5
# speedup vs baseline: 1.0041x; 1.0041x over previous
"""Trainium2 kernel for nn_CuriosityEngine.

The reference broadcasts x to 32 identical "action replicas" before the two
EntangledBottleneck passes, so the 32 per-action entropies are bitwise
identical and argmax over them is always 0, for any input.  The only
input-dependent output is `surprise`:

    state_mean = x.mean(axis=1)                      # (B, D, 2): reads all of x
    surprise   = relu(state_mean.flat @ W1 + b1) @ W2 + b2

The memory roofline is one pass over x (48 MB -> 6 MB/core on 8 cores).

Sharding: x.reshape(8192, 1536) is split into 8 contiguous 1024-row blocks
(core c's rows all belong to batch b = c // 2), so shards are zero-copy views
and the host just adds core-pair partials.

Per-core kernel (raw Bass, hand-scheduled; ~24.8 us cost-model, ~12 us/body
measured on HW via replication slope — DMA-bound):
  - 16 HWDGE DMAs on one queue: 8 tiles split into L (1024 cols) then
    R (512 cols) streams; in-order completion per queue.
  - DVE sequentially accumulates tiles into accL/accR as they land (the adds
    hide under the DMA stream; per-tile semaphores + a retired-add counter
    keep every semaphore edge deterministic).
  - GpSimd partition_all_reduce folds the 128 partitions; the big L reduce
    hides under the R DMA stream, so the tail after the last byte is just
    small-R add -> small-R reduce -> one 6 KB out-DMA.
Host: sum core-pair partials in f64, divide by T, run the 1.5 MFLOP MLP.
"""

import contextlib

import numpy as np

import concourse.bass as bass
import concourse.bacc as bacc
import concourse.bass_isa as bass_isa
import concourse.mybir as mybir
from concourse import library_config
from concourse.bass_utils import run_bass_kernel_spmd

N_CORES = 8
B, T, D = 4, 2048, 768
F = 2 * D            # 1536
ROWS = (B * T) // N_CORES  # 1024 rows per core
P = 128
N_TILES = ROWS // P  # 8
N_ADDS = 2 * (N_TILES - 1) + 1  # last R add split into two halves
HL = 1024            # left-half columns (bulk)
HR = F - HL          # 512 right-half columns (tail chain runs on R)
HQ = HR // 2         # final R tile lands as two 256-col chunks
F32 = mybir.dt.float32

TRACE = False
LAST_RESULTS = None
_NC = None


def _build():
    nc = bacc.Bacc("TRN2", num_devices=N_CORES)
    xs = nc.dram_tensor("xs", [ROWS, F], F32, kind="ExternalInput")
    out = nc.dram_tensor("partial", [1, F], F32, kind="ExternalOutput")

    with contextlib.ExitStack() as ctx:
        xtL = [ctx.enter_context(nc.sbuf_tensor(f"xtL{i}", [P, HL], F32))
               for i in range(N_TILES)]
        xtR = [ctx.enter_context(nc.sbuf_tensor(f"xtR{i}", [P, HR], F32))
               for i in range(N_TILES)]
        accL = ctx.enter_context(nc.sbuf_tensor("accL", [P, HL], F32))
        accR = ctx.enter_context(nc.sbuf_tensor("accR", [P, HR], F32))
        red = ctx.enter_context(nc.sbuf_tensor("red", [P, F], F32))
        semL = [ctx.enter_context(nc.semaphore(f"semL{i}"))
                for i in range(N_TILES)]
        semR = [ctx.enter_context(nc.semaphore(f"semR{i}"))
                for i in range(N_TILES)]
        semR7b = ctx.enter_context(nc.semaphore("semR7b"))
        dve_sem = ctx.enter_context(nc.semaphore("dve_sem"))
        red_sem = ctx.enter_context(nc.semaphore("red_sem"))
        out_sem = ctx.enter_context(nc.semaphore("out_sem"))
        block = ctx.enter_context(nc.Block())

        @block.sync
        def _(sync):
            for i in range(N_TILES):
                sync.dma_start(
                    xtL[i][:], xs[i * P:(i + 1) * P, 0:HL]
                ).then_inc(semL[i], 16)
            for i in range(N_TILES - 1):
                sync.dma_start(
                    xtR[i][:], xs[i * P:(i + 1) * P, HL:F]
                ).then_inc(semR[i], 16)
            i = N_TILES - 1
            sync.dma_start(
                xtR[i][:, 0:HQ], xs[i * P:(i + 1) * P, HL:HL + HQ]
            ).then_inc(semR[i], 16)
            sync.dma_start(
                xtR[i][:, HQ:HR], xs[i * P:(i + 1) * P, HL + HQ:F]
            ).then_inc(semR7b, 16)
            sync.wait_ge(red_sem, 3)
            sync.dma_start(out[0:1, :], red[0:1, :]).then_inc(out_sem, 16)
            sync.wait_ge(out_sem, 16)

        @block.vector
        def _(vector):
            # L chain then R chain; dve_sem counts retired adds (the DVE
            # pipeline has no RAW interlock, so each add waits for its
            # predecessor to retire).
            k = 0
            vector.wait_ge(semL[0], 16)
            vector.wait_ge(semL[1], 16)
            vector.tensor_add(accL[:], xtL[0][:], xtL[1][:]).then_inc(dve_sem, 1)
            k += 1
            for i in range(2, N_TILES):
                vector.wait_ge(semL[i], 16)
                vector.wait_ge(dve_sem, k)
                vector.tensor_add(accL[:], accL[:], xtL[i][:]).then_inc(dve_sem, 1)
                k += 1
            vector.wait_ge(semR[0], 16)
            vector.wait_ge(semR[1], 16)
            vector.tensor_add(accR[:], xtR[0][:], xtR[1][:]).then_inc(dve_sem, 1)
            k += 1
            for i in range(2, N_TILES - 1):
                vector.wait_ge(semR[i], 16)
                vector.wait_ge(dve_sem, k)
                vector.tensor_add(accR[:], accR[:], xtR[i][:]).then_inc(dve_sem, 1)
                k += 1
            i = N_TILES - 1
            vector.wait_ge(semR[i], 16)
            vector.wait_ge(dve_sem, k)
            vector.tensor_add(
                accR[:, 0:HQ], accR[:, 0:HQ], xtR[i][:, 0:HQ]
            ).then_inc(dve_sem, 1)
            k += 1
            vector.wait_ge(semR7b, 16)
            vector.wait_ge(dve_sem, k)
            vector.tensor_add(
                accR[:, HQ:HR], accR[:, HQ:HR], xtR[i][:, HQ:HR]
            ).then_inc(dve_sem, 1)
            k += 1

        @block.gpsimd
        def _(gpsimd):
            gpsimd.load_library(library_config.mlp)
            gpsimd.wait_ge(dve_sem, N_TILES - 1)
            gpsimd.partition_all_reduce(
                red[:, 0:HL], accL[:], channels=P,
                reduce_op=bass_isa.ReduceOp.add,
            ).then_inc(red_sem, 1)
            gpsimd.wait_ge(dve_sem, N_ADDS - 1)
            gpsimd.partition_all_reduce(
                red[:, HL:HL + HQ], accR[:, 0:HQ], channels=P,
                reduce_op=bass_isa.ReduceOp.add,
            ).then_inc(red_sem, 1)
            gpsimd.wait_ge(dve_sem, N_ADDS)
            gpsimd.partition_all_reduce(
                red[:, HL + HQ:F], accR[:, HQ:HR], channels=P,
                reduce_op=bass_isa.ReduceOp.add,
            ).then_inc(red_sem, 1)

    nc.compile()
    return nc


def _get_nc():
    global _NC
    if _NC is None:
        _NC = _build()
    return _NC


def kernel(x, W1, b1, W2, b2, Wd, bd, Wu, bu):
    global LAST_RESULTS
    x = np.ascontiguousarray(np.asarray(x, dtype=np.float32))
    W1 = np.asarray(W1, dtype=np.float32)
    b1 = np.asarray(b1, dtype=np.float32)
    W2 = np.asarray(W2, dtype=np.float32)
    b2 = np.asarray(b2, dtype=np.float32)

    x_flat = x.reshape(B * T, F)
    in_maps = [{"xs": x_flat[c * ROWS:(c + 1) * ROWS]} for c in range(N_CORES)]

    nc = _get_nc()
    res = run_bass_kernel_spmd(nc, in_maps, list(range(N_CORES)), trace=TRACE)
    LAST_RESULTS = res

    partials = np.stack([r["partial"][0] for r in res.results])  # (8, 1536)
    sums = partials.astype(np.float64).reshape(B, 2, F).sum(axis=1)
    state_mean = (sums / T).astype(np.float32)                   # (B, 1536)

    h = np.maximum(state_mean @ W1 + b1, 0.0)
    surprise = (h @ W2 + b2).astype(np.float32).reshape(B)

    best_action_idx = np.array(0, dtype=np.int32)
    return surprise, best_action_idx
